# revision 29
# baseline (speedup 1.0000x reference)
"""Distance-loss kernel for Trainium2 (8 NeuronCores, data-parallel over batch).

loss = mean over (b, c != label_b) of sqrt(||x_b - center_c||^2)

Sharding/layout (host side): x and labels are sharded over batch; centers
are replicated. x and centers are additionally staged in d-major layout
(xT, cT) so the device needs no transposes — pure layout staging, all
arithmetic stays on device.

Per-core plan (B_shard = 2048 rows, distmat computed as out[c, b]):
  - psum[c, b] = -2 * c_c . x_b + ||x_b||^2 via PE matmuls in bf16 (fp32
    matmul streams at ~1/4 rate on trn2). Centers are the stationary
    operand, so one LDWEIGHTS serves 4 matmuls (weight switching every
    matmul keeps the PE clock-gated cold). The row-norm term rides a K=2
    augmented matmul as a double-bf16 (hi+lo) pair -> fp32-level accuracy.
  - d = sqrt(psum + ||c_c||^2): class norm, exact fp32, as the ScalarE
    per-partition bias; the same instruction accumulates sum_b d.
  - label-entry correction: gather centers[labels] with one dma_gather,
    sum_d (x-g)^2 on VectorE, sqrt once at the end, subtract.
  - host sums the 8 per-core partials and divides by B*(C-1).
"""

import sys
from contextlib import ExitStack

import numpy as np

if "/opt/trn_rl_repo" not in sys.path:
    sys.path.insert(0, "/opt/trn_rl_repo")

import concourse.bass as bass
import concourse.mybir as mybir
from concourse.bacc import Bacc
from concourse.bass import IndirectOffsetOnAxis
from concourse.masks import make_identity
from concourse.tile import TileContext

F32 = mybir.dt.float32
BF16 = mybir.dt.bfloat16
I16 = mybir.dt.int16
I32 = mybir.dt.int32
AF = mybir.ActivationFunctionType
ALU = mybir.AluOpType

N_CORES = 8
B = 16384
C = 1000
D = 256
BS = B // N_CORES          # 2048 rows per core
T = BS // 128              # 16 b-tiles per core
NC_TILES = 8               # ceil(C / 128) class tiles
AUG_LAG = 2                # c-tiles between k-matmuls and their aug/ACT


def build_nc() -> bass.Bass:
    nc = Bacc()
    x_d = nc.dram_tensor("x", [BS, D], F32, kind="ExternalInput")
    xT_d = nc.dram_tensor("xT", [D, BS], F32, kind="ExternalInput")
    c_d = nc.dram_tensor("centers", [C, D], F32, kind="ExternalInput")
    cT_d = nc.dram_tensor("cT", [D, C], F32, kind="ExternalInput")
    l_d = nc.dram_tensor("labels", [128, T], I32, kind="ExternalInput")
    o_d = nc.dram_tensor("out", [1, 1], F32, kind="ExternalOutput")

    with TileContext(nc) as tc, ExitStack() as ctx:
        const = ctx.enter_context(tc.tile_pool(name="const", bufs=1))
        setup_sb = ctx.enter_context(tc.tile_pool(name="setup_sb", bufs=2))
        xpool = ctx.enter_context(tc.tile_pool(name="xpool", bufs=3))
        dpool = ctx.enter_context(tc.tile_pool(name="dpool", bufs=2))
        xps = ctx.enter_context(tc.tile_pool(name="xps", bufs=2, space="PSUM"))
        mmps = ctx.enter_context(tc.tile_pool(name="mmps", bufs=3, space="PSUM"))

        ident = const.tile([128, 128], F32)
        make_identity(nc, ident[:])

        # PE warm-up burst: dense same-weight matmuls while DMAs stream in,
        # so the HAM clock gate reaches 2.4 GHz before the real work.
        wu_w = const.tile([128, 128], BF16)
        nc.vector.memset(wu_w[:], 0.5)
        wu_r = const.tile([128, 512], BF16)
        nc.vector.memset(wu_r[:], 0.25)
        wu_ps = xps.tile([128, 512], F32, tag="xps")
        for rep in range(18):
            nc.tensor.matmul(wu_ps[:, :], wu_w[:], wu_r[:],
                             start=(rep == 0), stop=(rep == 17))
        wu_out = const.tile([1, 1], F32)
        nc.scalar.copy(wu_out[:], wu_ps[0:1, 0:1])

        # -2 * centers^T in bf16 (stationary operand)
        cTb0 = const.tile([128, C], BF16, tag="cTb0")
        cTb1 = const.tile([128, C], BF16, tag="cTb1")
        cTb = [cTb0, cTb1]
        ctf = []
        for k in range(2):
            ctfk = setup_sb.tile([128, C], F32, tag=f"ctf{k}")
            nc.sync.dma_start(out=ctfk[:], in_=cT_d[k * 128 : (k + 1) * 128, :])
            ctf.append(ctfk)
        for k in range(2):
            nc.vector.tensor_scalar_mul(cTb[k][:], ctf[k][:], -2.0)

        # x^T in bf16 (moving operand), [128,512] tiles for fine-grained deps
        xTb = [[None] * 4, [None] * 4]
        xtf = []
        for k in range(2):
            xtfk = setup_sb.tile([128, BS], F32, tag=f"xtf{k}")
            nc.sync.dma_start(out=xtfk[:], in_=xT_d[k * 128 : (k + 1) * 128, :])
            xtf.append(xtfk)
        # natural-layout x resident, 4 chunk tiles (second DMA queue)
        x_sb = []
        for q in range(4):
            x_q = const.tile([128, 4 * D], F32, tag=f"x_sb{q}")
            nc.scalar.dma_start(
                out=x_q[:].rearrange("p (t d) -> p t d", d=D),
                in_=x_d[q * 512 : (q + 1) * 512, :].rearrange(
                    "(t p) d -> p t d", p=128),
            )
            x_sb.append(x_q)

        def x_slice(t):
            return x_sb[t // 4][:, (t % 4) * D : (t % 4 + 1) * D]

        # interleave xTb casts with xx STTs so neither starves the PE
        xxP = const.tile([128, T], F32)
        for j in range(4):
            for k in range(2):
                xb = const.tile([128, 512], BF16, tag=f"xTb{k}_{j}")
                nc.vector.tensor_copy(xb[:], xtf[k][:, j * 512 : (j + 1) * 512])
                xTb[k][j] = xb
            for t in range(j * 4, j * 4 + 4):
                xsq = xpool.tile([128, D], F32, tag="xsq")
                nc.vector.scalar_tensor_tensor(
                    out=xsq[:], in0=x_slice(t), scalar=0.0, in1=x_slice(t),
                    op0=ALU.bypass, op1=ALU.mult,
                    accum_out=xxP[:, t : t + 1],
                )

        # ||x||^2 rows -> double-bf16 rows xx2 [2, BS] for the aug matmul
        xxps = xps.tile([128, 512], F32, tag="xps")
        nc.tensor.transpose(xxps[0:T, 0:128], xxP[:, :], ident[:])
        xxT16 = const.tile([T, 128], F32)
        nc.vector.tensor_copy(xxT16[:], xxps[0:T, 0:128])
        xxhi = const.tile([T, 128], BF16)
        nc.vector.tensor_copy(xxhi[:], xxT16[:])
        xxrem = const.tile([T, 128], F32)
        nc.vector.tensor_sub(xxrem[:], xxT16[:], xxhi[:])
        xxlo = const.tile([T, 128], BF16)
        nc.vector.tensor_copy(xxlo[:], xxrem[:])
        xx2 = const.tile([2, BS], BF16)
        xxhl_d = nc.dram_tensor("xxhl_scratch", [2, T, 128], BF16)
        nc.scalar.dma_start(out=xxhl_d[0, :, :], in_=xxhi[:, :])
        nc.scalar.dma_start(out=xxhl_d[1, :, :], in_=xxlo[:, :])
        nc.scalar.dma_start(
            out=xx2[:, :], in_=xxhl_d[:, :, :].rearrange("o t p -> o (t p)")
        )
        ones2 = const.tile([2, 128], BF16)
        nc.vector.memset(ones2[:], 1.0)

        # ||c||^2 per class (fp32, class-major -> ACT bias columns)
        ccP = const.tile([128, NC_TILES], F32)
        for i in range(NC_TILES):
            r0 = i * 128
            cnt = min(128, C - r0)
            ct = setup_sb.tile([cnt, D], F32, tag="ctile")
            nc.scalar.dma_start(out=ct[:], in_=c_d[r0 : r0 + cnt, :])
            csq = setup_sb.tile([cnt, D], F32, tag="csq")
            nc.vector.scalar_tensor_tensor(
                out=csq[:], in0=ct[:], scalar=0.0, in1=ct[:],
                op0=ALU.bypass, op1=ALU.mult,
                accum_out=ccP[0:cnt, i : i + 1],
            )

        # gather g[p, t*D:(t+1)*D] = centers[labels[t*128+p]], one indirect
        # DMA per b-tile ([128,1] per-partition offsets)
        lab_sb = const.tile([128, T], I32)
        nc.scalar.dma_start(out=lab_sb[:], in_=l_d[:, :])
        g_sb = const.tile([128, T * D], F32)
        for t in range(T):
            nc.gpsimd.indirect_dma_start(
                out=g_sb[:, t * D : (t + 1) * D],
                out_offset=None,
                in_=c_d[:, :],
                in_offset=IndirectOffsetOnAxis(ap=lab_sb[:, t : t + 1], axis=0),
            )

        acc = const.tile([128, 2 * NC_TILES], F32)   # sum_b sqrt(dist)
        dacc = const.tile([128, T], F32)             # label-entry dist^2
        nc.vector.memset(acc[:], 0.0)

        # main c-tile loop; aug matmuls + ACT trail by AUG_LAG c-tiles so a
        # late xx2 doesn't stall the in-order PE queue.
        pending = []

        def finish_ctile(m, psA, psB):
            cnt = min(128, C - m * 128)
            for j, pst in ((0, psA), (1, psA), (2, psB), (3, psB)):
                nc.tensor.matmul(
                    pst[0 : cnt, (j % 2) * 512 : (j % 2) * 512 + 512],
                    ones2[:, 0:cnt],
                    xx2[:, j * 512 : (j + 1) * 512],
                    start=False, stop=True,
                )
            for h, pst in ((0, psA), (1, psB)):
                dt_ = dpool.tile([128, 1024], F32, tag="d")
                nc.scalar.activation(
                    dt_[0:cnt, :], pst[0:cnt, :], AF.Sqrt,
                    bias=ccP[0:cnt, m : m + 1], scale=1.0,
                    accum_out=acc[0:cnt, 2 * m + h : 2 * m + h + 1],
                )

        for m in range(NC_TILES):
            cnt = min(128, C - m * 128)
            msl = slice(m * 128, m * 128 + cnt)
            psA = mmps.tile([128, 1024], F32, tag="mm")
            psB = mmps.tile([128, 1024], F32, tag="mm")
            for k in range(2):
                for j, pst in ((0, psA), (1, psA), (2, psB), (3, psB)):
                    nc.tensor.matmul(
                        pst[0 : cnt, (j % 2) * 512 : (j % 2) * 512 + 512],
                        cTb[k][:, msl],
                        xTb[k][j][:],
                        start=(k == 0), stop=False,
                    )
            pending.append((m, psA, psB))
            if len(pending) > AUG_LAG:
                finish_ctile(*pending.pop(0))
        while pending:
            finish_ctile(*pending.pop(0))

        # label-entry correction
        for t in range(T):
            df = xpool.tile([128, D], BF16, tag="df")
            nc.vector.tensor_sub(df[:], x_slice(t),
                                 g_sb[:, t * D : (t + 1) * D])
            dfsq = xpool.tile([128, D], BF16, tag="dfsq")
            nc.vector.scalar_tensor_tensor(
                out=dfsq[:], in0=df[:], scalar=0.0, in1=df[:],
                op0=ALU.bypass, op1=ALU.mult, accum_out=dacc[:, t : t + 1],
            )

        corr_s = const.tile([128, T], F32)
        nc.scalar.activation(corr_s[:], dacc[:], AF.Sqrt)
        totp = const.tile([128, 1], F32)
        corp = const.tile([128, 1], F32)
        nc.vector.reduce_sum(out=totp[:], in_=acc[:], axis=mybir.AxisListType.X)
        nc.vector.reduce_sum(out=corp[:], in_=corr_s[:], axis=mybir.AxisListType.X)
        part = const.tile([128, 1], F32)
        nc.vector.tensor_sub(part[:], totp[:], corp[:])
        ones_col = const.tile([128, 1], F32)
        nc.any.memset(ones_col[:], 1.0)
        red_ps = xps.tile([1, 1], F32, tag="xps")
        nc.tensor.matmul(red_ps[0:1, 0:1], ones_col[:], part[:],
                         start=True, stop=True)
        red = const.tile([1, 1], F32)
        nc.scalar.copy(red[:], red_ps[0:1, 0:1])
        nc.sync.dma_start(out=o_d[0:1, 0:1], in_=red[0:1, 0:1])

    nc.compile()
    return nc


_NC_CACHE = None


def _get_nc():
    global _NC_CACHE
    if _NC_CACHE is None:
        _NC_CACHE = build_nc()
    return _NC_CACHE


def make_in_maps(x, centers, labels):
    x = np.ascontiguousarray(np.asarray(x, dtype=np.float32))
    centers = np.ascontiguousarray(np.asarray(centers, dtype=np.float32))
    cT = np.ascontiguousarray(centers.T)
    labels = np.asarray(labels)
    in_maps = []
    for i in range(N_CORES):
        xs = x[i * BS : (i + 1) * BS]
        xT = np.ascontiguousarray(xs.T)
        ls = labels[i * BS : (i + 1) * BS].astype(np.int32)
        # lab[p, t] = label of shard row t*128 + p (indirect-gather order)
        lab = np.ascontiguousarray(ls.reshape(T, 128).T)
        in_maps.append({"x": xs, "xT": xT, "centers": centers, "cT": cT,
                        "labels": lab})
    return in_maps


def _ensure_ntff_hook_module():
    """Provide antenv.axon_hooks if the image's antenv package lacks it.

    concourse.bass_utils imports it for trace=True under axon; the hook
    itself lives in libaxon_pjrt.so and is wrapped by trn_agent_boot.
    """
    import types

    try:
        import antenv.axon_hooks  # noqa: F401
        return
    except ImportError:
        pass
    mod = types.ModuleType("antenv.axon_hooks")
    state = {"hook": None}

    def set_axon_ntff_profile_hook(hook):
        state["hook"] = hook

    def get_axon_ntff_profile_hook():
        if state["hook"] is None:
            try:
                from trn_agent_boot.trn_boot import _ntff_profile_via_ctypes

                state["hook"] = _ntff_profile_via_ctypes(
                    "/opt/axon/libaxon_pjrt.so"
                )
            except Exception:
                return None
        return state["hook"]

    mod.set_axon_ntff_profile_hook = set_axon_ntff_profile_hook
    mod.get_axon_ntff_profile_hook = get_axon_ntff_profile_hook
    sys.modules["antenv.axon_hooks"] = mod
    try:
        import antenv

        antenv.axon_hooks = mod
    except ImportError:
        pass


def kernel(x, centers, labels, _results_out=None, **run_kwargs):
    _ensure_ntff_hook_module()
    from concourse.bass_utils import run_bass_kernel_spmd

    nc = _get_nc()
    in_maps = make_in_maps(x, centers, labels)
    res = run_bass_kernel_spmd(nc, in_maps, core_ids=list(range(N_CORES)),
                               **run_kwargs)
    if _results_out is not None:
        _results_out.append(res)
    partials = [float(r["out"][0, 0]) for r in res.results]
    total = float(np.sum(np.asarray(partials, dtype=np.float64)))
    loss = total / (B * (C - 1))
    return np.float32(loss)
